# revision 1
# baseline (speedup 1.0000x reference)
"""LoRALinear kernel for Trainium2 (8 NeuronCores, data-parallel over tokens).

Math: out = x @ W.T + b + s1*(x@A1.T)@B1.T + s2*(x@A2.T)@B2.T
    = x @ (W + s1*B1@A1 + s2*B2@A2).T + b

The LoRA adapters are folded into the base weight on-device (rank-16 fold is
tiny), turning the whole problem into one dense [T,1024]@[1024,1024] matmul
plus a broadcast bias add. x is sharded 4096 tokens per core; all weights are
replicated; no collectives.

Sharding/layout choice (host side, pure layout transforms only): x is passed
per-core as x.T columns so the contraction dim lands on SBUF partitions with
fast contiguous DMA; W/B1/B2 are passed transposed for the same reason. All
arithmetic (scaling, LoRA fold, matmul, bias) runs on device.

Per-core pipeline:
  prep : DMA W.T, round to fp32r (DVE), fold s1*A1.T@B1.T + s2*A2.T@B2.T via
         two rank-16 PE matmuls per tile + DVE add; DMA-broadcast bias.
  main : per 128-token tile: DMA xT tile, DVE fp32r rounding copy,
         8 accumulating fp32r matmuls per 512-wide psum, DVE bias-add, DMA out.
"""

import sys

import numpy as np

try:
    import concourse.bass as bass
except ImportError:
    sys.path.insert(0, "/opt/trn_rl_repo")
    import concourse.bass as bass

from concourse import bacc

import concourse.mybir as mybir
import concourse.tile as tile
from concourse.bass_utils import run_bass_kernel_spmd

TOKENS, D, RANK = 32768, 1024, 16
N_CORES = 8
T_SHARD = TOKENS // N_CORES  # 4096
SCALE1 = 8.0 / RANK
SCALE2 = 16.0 / RANK
F32 = mybir.dt.float32
F32R = mybir.dt.float32r
P = 128
N_TT = T_SHARD // P  # 32 token tiles per core
N_IC = D // P  # 8 contraction chunks
N_OC = D // 512  # 2 psum-wide output chunks


def build_nc():
    nc = bacc.Bacc("TRN2")
    xT = nc.dram_tensor("xT", [D, T_SHARD], F32, kind="ExternalInput")
    WT = nc.dram_tensor("WT", [D, D], F32, kind="ExternalInput")
    b = nc.dram_tensor("b", [D], F32, kind="ExternalInput")
    A1 = nc.dram_tensor("A1", [RANK, D], F32, kind="ExternalInput")
    B1T = nc.dram_tensor("B1T", [RANK, D], F32, kind="ExternalInput")
    A2 = nc.dram_tensor("A2", [RANK, D], F32, kind="ExternalInput")
    B2T = nc.dram_tensor("B2T", [RANK, D], F32, kind="ExternalInput")
    out = nc.dram_tensor("out", [T_SHARD, D], F32, kind="ExternalOutput")

    with tile.TileContext(nc) as tc:
        with (
            tc.tile_pool(name="const", bufs=1) as const,
            tc.tile_pool(name="xp", bufs=4) as xpool,
            tc.tile_pool(name="xtp", bufs=4) as xtpool,
            tc.tile_pool(name="op", bufs=4) as opool,
            tc.tile_pool(name="psm", bufs=4, space="PSUM") as psum_m,
        ):
            # bias broadcast across all 128 partitions (tokens sit on partitions)
            bias_sb = const.tile([P, D], F32)
            b_ap = b[:]
            bias_bcast = bass.AP(
                tensor=b_ap.tensor, offset=b_ap.offset, ap=[[0, P], [1, D]]
            )
            nc.sync.dma_start(out=bias_sb, in_=bias_bcast)

            # W.T layout [i_inner(128), i_outer(8), o(1024)], rounded to fp32r
            WT_ld = const.tile([P, N_IC, D], F32)
            nc.sync.dma_start(WT_ld, WT[:].rearrange("(io ii) o -> ii io o", ii=P))
            WT_sb = const.tile([P, N_IC, D], F32R)
            for io in range(N_IC):
                nc.vector.tensor_copy(out=WT_sb[:, io, :], in_=WT_ld[:, io, :])

            # adapters (A natural, B pre-transposed on host; scales on device)
            A1_ld = const.tile([RANK, D], F32)
            nc.sync.dma_start(A1_ld, A1[:])
            A2_ld = const.tile([RANK, D], F32)
            nc.sync.dma_start(A2_ld, A2[:])
            A1_sb = const.tile([RANK, D], F32R)
            nc.vector.tensor_copy(out=A1_sb, in_=A1_ld)
            A2_sb = const.tile([RANK, D], F32R)
            nc.vector.tensor_copy(out=A2_sb, in_=A2_ld)

            B1T_ld = const.tile([RANK, D], F32)
            nc.sync.dma_start(B1T_ld, B1T[:])
            B2T_ld = const.tile([RANK, D], F32)
            nc.sync.dma_start(B2T_ld, B2T[:])
            B1T_sb = const.tile([RANK, D], F32R)
            nc.vector.tensor_scalar_mul(B1T_sb, B1T_ld, SCALE1)
            B2T_sb = const.tile([RANK, D], F32R)
            nc.vector.tensor_scalar_mul(B2T_sb, B2T_ld, SCALE2)

            # fold LoRA: WT += s1*A1.T@B1.T + s2*A2.T@B2.T (scales baked in BT)
            for ic in range(N_IC):
                for on in range(N_OC):
                    psd = psum_m.tile([P, 512], F32, tag="psd")
                    nc.tensor.matmul(
                        psd,
                        lhsT=A1_sb[:, ic * P : (ic + 1) * P],
                        rhs=B1T_sb[:, on * 512 : (on + 1) * 512],
                        start=True,
                        stop=False,
                    )
                    nc.tensor.matmul(
                        psd,
                        lhsT=A2_sb[:, ic * P : (ic + 1) * P],
                        rhs=B2T_sb[:, on * 512 : (on + 1) * 512],
                        start=False,
                        stop=True,
                    )
                    nc.vector.tensor_add(
                        out=WT_sb[:, ic, on * 512 : (on + 1) * 512],
                        in0=WT_sb[:, ic, on * 512 : (on + 1) * 512].bitcast(F32),
                        in1=psd,
                    )

            # main loop: 32 token tiles of 128
            for tt in range(N_TT):
                x_ld = xpool.tile([P, N_IC, P], F32, tag="x")
                nc.sync.dma_start(
                    x_ld,
                    xT[:, tt * P : (tt + 1) * P].rearrange(
                        "(io ii) t -> ii io t", ii=P
                    ),
                )
                xT_sb = xtpool.tile([P, N_IC, P], F32R, tag="xt")
                nc.vector.tensor_copy(out=xT_sb, in_=x_ld)
                o_sb = opool.tile([P, D], F32, tag="o")
                for on in range(N_OC):
                    pso = psum_m.tile([P, 512], F32, tag="psd")
                    for ic in range(N_IC):
                        nc.tensor.matmul(
                            pso,
                            lhsT=xT_sb[:, ic, :],
                            rhs=WT_sb[:, ic, on * 512 : (on + 1) * 512],
                            start=(ic == 0),
                            stop=(ic == N_IC - 1),
                        )
                    nc.vector.tensor_add(
                        out=o_sb[:, on * 512 : (on + 1) * 512],
                        in0=pso,
                        in1=bias_sb[:, on * 512 : (on + 1) * 512],
                    )
                nc.sync.dma_start(out[tt * P : (tt + 1) * P, :], o_sb)

    nc.finalize()
    return nc


_NC = None


def _get_nc():
    global _NC
    if _NC is None:
        _NC = build_nc()
    return _NC


def kernel(**inputs):
    x = np.asarray(inputs["x"], dtype=np.float32)
    shared = {
        "WT": np.ascontiguousarray(np.asarray(inputs["W"], np.float32).T),
        "b": np.ascontiguousarray(np.asarray(inputs["b"], np.float32)),
        "A1": np.ascontiguousarray(np.asarray(inputs["A1"], np.float32)),
        "B1T": np.ascontiguousarray(np.asarray(inputs["B1"], np.float32).T),
        "A2": np.ascontiguousarray(np.asarray(inputs["A2"], np.float32)),
        "B2T": np.ascontiguousarray(np.asarray(inputs["B2"], np.float32).T),
    }
    in_maps = []
    for c in range(N_CORES):
        m = dict(shared)
        m["xT"] = np.ascontiguousarray(x[c * T_SHARD : (c + 1) * T_SHARD].T)
        in_maps.append(m)
    res = run_bass_kernel_spmd(_get_nc(), in_maps, core_ids=list(range(N_CORES)))
    return np.concatenate([r["out"] for r in res.results], axis=0)



# revision 6
# speedup vs baseline: 1.1498x; 1.1498x over previous
"""LoRALinear kernel for Trainium2 (8 NeuronCores, data-parallel over tokens).

Math: out = x @ W.T + b + s1*(x@A1.T)@B1.T + s2*(x@A2.T)@B2.T
    = x @ (W + s1*B1@A1 + s2*B2@A2).T + b

The two rank-16 adapters are stacked into one rank-32 pair (host-side
concatenation, layout only) and folded into the base weight on-device,
turning the problem into one dense [T,1024]@[1024,1024] matmul plus a
broadcast bias add. x is sharded 4096 tokens per core; weights replicated.

v2 changes vs v1:
  - No fp32->fp32r DVE conversion copies: matmul operands are bitcast to
    fp32r in place (same bits, PE's 1-pass fp32 mode).
  - W is DMA'd in 8 contraction chunks; the LoRA fold for each chunk runs
    as soon as that chunk + the (tiny) adapters arrive, interleaved with
    x-chunk prefetch, so the PE starts real work ~4us in.
  - PE warmup matmuls on a zeroed scratch tile absorb the slow-clock ramp
    while the first DMAs are still in flight.
  - Input DMAs issue on SP, output DMAs on ACT: a store waiting for its
    tile can't head-of-line block the input prefetch stream.
"""

import sys

import numpy as np

try:
    import concourse.bass as bass
except ImportError:
    sys.path.insert(0, "/opt/trn_rl_repo")
    import concourse.bass as bass

from concourse import bacc

import concourse.mybir as mybir
import concourse.tile as tile
from concourse.bass_utils import run_bass_kernel_spmd

TOKENS, D, RANK = 32768, 1024, 16
RANK2 = 2 * RANK
N_CORES = 8
T_SHARD = TOKENS // N_CORES  # 4096
SCALE1 = 8.0 / RANK
SCALE2 = 16.0 / RANK
F32 = mybir.dt.float32
F32R = mybir.dt.float32r
BF16 = mybir.dt.bfloat16
P = 128
N_IC = D // P  # 8 contraction chunks
N_OC = D // 512  # 2 psum-wide output chunks
N_TT = T_SHARD // P  # 32 token tiles per core
COPY = mybir.ActivationFunctionType.Copy

# x prefetch chunk sizes (tokens); first ones small so the PE can start early
X_CHUNKS = [128, 128, 256, 512, 512, 512, 512, 512, 512, 512]
assert sum(X_CHUNKS) == T_SHARD

N_WARM = 7  # PE warmup matmuls; end right when fold0's inputs land


def build_nc():
    nc = bacc.Bacc("TRN2")
    xT = nc.dram_tensor("xT", [D, T_SHARD], F32R, kind="ExternalInput")
    WT = nc.dram_tensor("WT", [D, D], F32R, kind="ExternalInput")
    b = nc.dram_tensor("b", [D], F32, kind="ExternalInput")
    A = nc.dram_tensor("A", [RANK2, D], F32R, kind="ExternalInput")
    BT = nc.dram_tensor("BT", [RANK2, D], F32, kind="ExternalInput")
    SC = nc.dram_tensor("SC", [RANK2, 1], F32, kind="ExternalInput")
    out = nc.dram_tensor("out", [T_SHARD, D], BF16, kind="ExternalOutput")

    with tile.TileContext(nc) as tc:
        with (
            tc.tile_pool(name="const", bufs=1) as const,
            tc.tile_pool(name="xp2", bufs=2) as xp2,
            tc.tile_pool(name="xp5", bufs=3) as xp5,
            tc.tile_pool(name="op", bufs=8) as opool,
            tc.tile_pool(name="psm", bufs=7, space="PSUM") as psum_m,
            tc.tile_pool(name="psf", bufs=1, space="PSUM") as psum_f,
        ):
            # --- PE warmup: matmuls on a zeroed scratch tile, results unused
            wz = const.tile([P, 512], F32)
            nc.vector.memset(wz, 0.0)
            warm = const.tile([P, 512], F32R)
            nc.vector.tensor_copy(out=warm, in_=wz)
            for i in range(N_WARM):
                pw = psum_f.tile([P, 512], F32, tag="fold")
                nc.tensor.matmul(
                    pw,
                    lhsT=warm[:, 0:P],
                    rhs=warm[:],
                    start=True,
                    stop=True,
                )

            # --- small constants: bias (partition-broadcast), adapters
            bias_sb = const.tile([P, D], F32)

            BT_ld = const.tile([RANK2, D], F32)
            nc.sync.dma_start(BT_ld, BT[:])
            A_sb = const.tile([RANK2, D], F32R)
            nc.sync.dma_start(A_sb, A[:])
            sc_sb = const.tile([RANK2, 1], F32)
            nc.sync.dma_start(sc_sb, SC[:])
            # one full-tile scale with a per-partition scalar vector
            # (partition-sliced engine ops must start at partition 0/32/64/96)
            BTs_sb = const.tile([RANK2, D], F32R)
            nc.scalar.activation(
                out=BTs_sb, in_=BT_ld, func=COPY, scale=sc_sb[:]
            )

            # --- W chunks + fold, interleaved with x prefetch.
            # One tile per contraction chunk; fold adds run on DVE in place.
            w_sb = [
                const.tile([P, D], F32R, name=f"w_sb{ic}") for ic in range(N_IC)
            ]

            x_tiles = []  # (tile, chunk_token_offset, n_tiles_in_chunk)

            def dma_w(ic):
                nc.sync.dma_start(w_sb[ic], WT[ic * P : (ic + 1) * P, :])
                for on in range(N_OC):
                    psd = psum_f.tile([P, 512], F32, tag="fold")
                    nc.tensor.matmul(
                        psd,
                        lhsT=A_sb[:, ic * P : (ic + 1) * P],
                        rhs=BTs_sb[:, on * 512 : (on + 1) * 512],
                        start=True,
                        stop=True,
                    )
                    nc.vector.tensor_add(
                        out=w_sb[ic][:, on * 512 : (on + 1) * 512],
                        in0=w_sb[ic][:, on * 512 : (on + 1) * 512].bitcast(F32),
                        in1=psd,
                    )

            def dma_x(ci, t0, tn):
                pool = xp2 if tn < 512 else xp5
                xt = pool.tile([P, N_IC, tn], F32R, tag=f"x{tn}")
                nc.sync.dma_start(
                    xt,
                    xT[:, t0 : t0 + tn].rearrange("(io ii) t -> ii io t", ii=P),
                )
                x_tiles.append((xt, t0, tn // P))

            # interleave: early x tiles feed partial psum groups while W streams
            t0 = 0
            ci = 0

            def next_x():
                nonlocal t0, ci
                dma_x(ci, t0, X_CHUNKS[ci])
                t0 += X_CHUNKS[ci]
                ci += 1

            dma_w(0)
            next_x()  # 128 tokens
            dma_w(1)
            next_x()  # 128 tokens
            dma_w(2)
            next_x()  # 256 tokens
            dma_w(3)
            dma_w(4)
            dma_w(5)
            b_ap = b[:]
            bias_bcast = bass.AP(
                tensor=b_ap.tensor, offset=b_ap.offset, ap=[[0, P], [1, D]]
            )
            nc.sync.dma_start(out=bias_sb, in_=bias_bcast)
            dma_w(6)
            dma_w(7)
            while ci < len(X_CHUNKS):
                next_x()

            # --- main loop: 128-token psum groups, 8 accumulating matmuls each
            last_tok = T_SHARD - P
            for xt, t0, ntile in x_tiles:
                for ti in range(ntile):
                    tok = t0 + ti * P
                    o_sb = opool.tile([P, D], BF16, tag="o")
                    is_last = tok == last_tok
                    for on in range(N_OC):
                        pso = psum_m.tile([P, 512], F32, tag="m")
                        for ic in range(N_IC):
                            nc.tensor.matmul(
                                pso,
                                lhsT=xt[:, ic, ti * P : (ti + 1) * P],
                                rhs=w_sb[ic][:, on * 512 : (on + 1) * 512],
                                start=(ic == 0),
                                stop=(ic == N_IC - 1),
                            )
                        nc.vector.tensor_add(
                            out=o_sb[:, on * 512 : (on + 1) * 512],
                            in0=pso,
                            in1=bias_sb[:, on * 512 : (on + 1) * 512],
                        )
                        if is_last:
                            # split the final store: each half leaves as soon
                            # as its bias add lands, on separate engines
                            eng = nc.scalar if on == 0 else nc.sync
                            eng.dma_start(
                                out[tok : tok + P, on * 512 : (on + 1) * 512],
                                o_sb[:, on * 512 : (on + 1) * 512],
                            )
                    if not is_last:
                        nc.scalar.dma_start(out[tok : tok + P, :], o_sb)

    nc.finalize()
    return nc


_NC = None


def _get_nc():
    global _NC
    if _NC is None:
        _NC = build_nc()
    return _NC


def kernel(**inputs):
    x = np.asarray(inputs["x"], dtype=np.float32)
    shared = {
        "WT": np.ascontiguousarray(np.asarray(inputs["W"], np.float32).T),
        "b": np.ascontiguousarray(np.asarray(inputs["b"], np.float32)),
        "A": np.ascontiguousarray(
            np.concatenate(
                [np.asarray(inputs["A1"], np.float32), np.asarray(inputs["A2"], np.float32)],
                axis=0,
            )
        ),
        "SC": np.asarray([SCALE1] * RANK + [SCALE2] * RANK, np.float32).reshape(
            RANK2, 1
        ),
        "BT": np.ascontiguousarray(
            np.concatenate(
                [np.asarray(inputs["B1"], np.float32).T, np.asarray(inputs["B2"], np.float32).T],
                axis=0,
            )
        ),
    }
    in_maps = []
    for c in range(N_CORES):
        m = dict(shared)
        m["xT"] = np.ascontiguousarray(x[c * T_SHARD : (c + 1) * T_SHARD].T)
        in_maps.append(m)
    res = run_bass_kernel_spmd(_get_nc(), in_maps, core_ids=list(range(N_CORES)))
    return np.concatenate(
        [np.asarray(r["out"]).astype(np.float32) for r in res.results], axis=0
    )


# revision 8
# speedup vs baseline: 1.1504x; 1.0005x over previous
"""LoRALinear kernel for Trainium2 (8 NeuronCores, data-parallel over tokens).

Math: out = x @ W.T + b + s1*(x@A1.T)@B1.T + s2*(x@A2.T)@B2.T
    = x @ (W + s1*B1@A1 + s2*B2@A2).T + b

The two rank-16 adapters are stacked into one rank-32 pair (host-side
concatenation, layout only) and folded into the base weight on-device,
turning the problem into one dense [T,1024]@[1024,1024] matmul plus a
broadcast bias add. x is sharded 4096 tokens per core; weights replicated.

Design notes (vs the v1 baseline, 148.3us -> 128.9us modeled):
  - No fp32->fp32r conversion copies: x/W/A live as fp32r end-to-end (the
    DMA moves the same fp32 bits; the PE consumes them in its 1-pass
    fp32r mode). Compute-produced matmul operands (scaled B^T, folded W)
    are written as fp32r by their producing op, which the BIR verifier
    requires ("rounded to FP32r").
  - W is DMA'd in 8 contraction chunks; each chunk's LoRA fold (rank-32
    matmul + in-place DVE add) runs as the chunk arrives, interleaved
    with the first x chunks, so the PE has work ~5us in. All transfers
    share one serial DMA pool, so the dma_start order below IS the
    schedule; it was tuned against the TimelineSim cost model.
  - PE warmup matmuls on a zeroed scratch tile absorb the slow-clock ramp
    while the first DMAs are in flight.
  - Per-partition-sliced engine ops must start at partition 0/32/64/96,
    so the two adapter scales (s1 rows 0..15, s2 rows 16..31) are one
    full-tile ACT op with a per-partition scale vector passed as a tiny
    constant input.
  - Output is stored as bf16 (adds ~1e-3 relative error against the 2e-2
    budget; halves store traffic) and exactly upcast on the host. Input
    DMAs issue on SP, stores on ACT, so a store waiting for its tile
    can't head-of-line block the input prefetch stream; the final store
    is split across both engines to shorten the tail.
"""

import sys

import numpy as np

try:
    import concourse.bass as bass
except ImportError:
    sys.path.insert(0, "/opt/trn_rl_repo")
    import concourse.bass as bass

from concourse import bacc

import concourse.mybir as mybir
import concourse.tile as tile
from concourse.bass_utils import run_bass_kernel_spmd

TOKENS, D, RANK = 32768, 1024, 16
RANK2 = 2 * RANK
N_CORES = 8
T_SHARD = TOKENS // N_CORES  # 4096
SCALE1 = 8.0 / RANK
SCALE2 = 16.0 / RANK
F32 = mybir.dt.float32
F32R = mybir.dt.float32r
BF16 = mybir.dt.bfloat16
P = 128
N_IC = D // P  # 8 contraction chunks
N_OC = D // 512  # 2 psum-wide output chunks
N_TT = T_SHARD // P  # 32 token tiles per core
COPY = mybir.ActivationFunctionType.Copy

# x prefetch chunk sizes (tokens); first ones small so the PE can start early
X_CHUNKS = [128, 128, 256, 256, 512, 512, 512, 512, 512, 512, 256]
assert sum(X_CHUNKS) == T_SHARD

N_WARM = 7  # PE warmup matmuls; end right when fold0's inputs land


def build_nc():
    nc = bacc.Bacc("TRN2")
    xT = nc.dram_tensor("xT", [D, T_SHARD], F32R, kind="ExternalInput")
    WT = nc.dram_tensor("WT", [D, D], F32R, kind="ExternalInput")
    b = nc.dram_tensor("b", [D], F32, kind="ExternalInput")
    A = nc.dram_tensor("A", [RANK2, D], F32R, kind="ExternalInput")
    BT = nc.dram_tensor("BT", [RANK2, D], F32, kind="ExternalInput")
    SC = nc.dram_tensor("SC", [RANK2, 1], F32, kind="ExternalInput")
    out = nc.dram_tensor("out", [T_SHARD, D], BF16, kind="ExternalOutput")

    with tile.TileContext(nc) as tc:
        with (
            tc.tile_pool(name="const", bufs=1) as const,
            tc.tile_pool(name="xp2", bufs=2) as xp2,
            tc.tile_pool(name="xp5", bufs=3) as xp5,
            tc.tile_pool(name="op", bufs=8) as opool,
            tc.tile_pool(name="psm", bufs=7, space="PSUM") as psum_m,
            tc.tile_pool(name="psf", bufs=1, space="PSUM") as psum_f,
        ):
            # --- PE warmup: matmuls on a zeroed scratch tile, results unused
            wz = const.tile([P, 512], F32)
            nc.vector.memset(wz, 0.0)
            warm = const.tile([P, 512], F32R)
            nc.vector.tensor_copy(out=warm, in_=wz)
            for i in range(N_WARM):
                pw = psum_f.tile([P, 512], F32, tag="fold")
                nc.tensor.matmul(
                    pw,
                    lhsT=warm[:, 0:P],
                    rhs=warm[:],
                    start=True,
                    stop=True,
                )

            # --- small constants: bias (partition-broadcast), adapters
            bias_sb = const.tile([P, D], F32)

            BT_ld = const.tile([RANK2, D], F32)
            nc.sync.dma_start(BT_ld, BT[:])
            A_sb = const.tile([RANK2, D], F32R)
            nc.sync.dma_start(A_sb, A[:])
            sc_sb = const.tile([RANK2, 1], F32)
            nc.sync.dma_start(sc_sb, SC[:])
            # one full-tile scale with a per-partition scalar vector
            # (partition-sliced engine ops must start at partition 0/32/64/96)
            BTs_sb = const.tile([RANK2, D], F32R)
            nc.scalar.activation(
                out=BTs_sb, in_=BT_ld, func=COPY, scale=sc_sb[:]
            )

            # --- W chunks + fold, interleaved with x prefetch.
            # One tile per contraction chunk; fold adds run on DVE in place.
            w_sb = [
                const.tile([P, D], F32R, name=f"w_sb{ic}") for ic in range(N_IC)
            ]

            x_tiles = []  # (tile, chunk_token_offset, n_tiles_in_chunk)

            def dma_w(ic):
                nc.sync.dma_start(w_sb[ic], WT[ic * P : (ic + 1) * P, :])
                for on in range(N_OC):
                    psd = psum_f.tile([P, 512], F32, tag="fold")
                    nc.tensor.matmul(
                        psd,
                        lhsT=A_sb[:, ic * P : (ic + 1) * P],
                        rhs=BTs_sb[:, on * 512 : (on + 1) * 512],
                        start=True,
                        stop=True,
                    )
                    nc.vector.tensor_add(
                        out=w_sb[ic][:, on * 512 : (on + 1) * 512],
                        in0=w_sb[ic][:, on * 512 : (on + 1) * 512].bitcast(F32),
                        in1=psd,
                    )

            def dma_x(ci, t0, tn):
                pool = xp2 if tn < 512 else xp5
                xt = pool.tile([P, N_IC, tn], F32R, tag=f"x{tn}")
                nc.sync.dma_start(
                    xt,
                    xT[:, t0 : t0 + tn].rearrange("(io ii) t -> ii io t", ii=P),
                )
                x_tiles.append((xt, t0, tn // P))

            # interleave: early x tiles feed partial psum groups while W streams
            t0 = 0
            ci = 0

            def next_x():
                nonlocal t0, ci
                dma_x(ci, t0, X_CHUNKS[ci])
                t0 += X_CHUNKS[ci]
                ci += 1

            dma_w(0)
            next_x()  # 128 tokens
            dma_w(1)
            next_x()  # 128 tokens
            dma_w(2)
            next_x()  # 256 tokens
            dma_w(3)
            dma_w(4)
            dma_w(5)
            b_ap = b[:]
            bias_bcast = bass.AP(
                tensor=b_ap.tensor, offset=b_ap.offset, ap=[[0, P], [1, D]]
            )
            nc.sync.dma_start(out=bias_sb, in_=bias_bcast)
            dma_w(6)
            dma_w(7)
            while ci < len(X_CHUNKS):
                next_x()

            # --- main loop: 128-token psum groups, 8 accumulating matmuls each
            last_tok = T_SHARD - P
            for xt, t0, ntile in x_tiles:
                for ti in range(ntile):
                    tok = t0 + ti * P
                    o_sb = opool.tile([P, D], BF16, tag="o")
                    is_last = tok == last_tok
                    for on in range(N_OC):
                        pso = psum_m.tile([P, 512], F32, tag="m")
                        for ic in range(N_IC):
                            nc.tensor.matmul(
                                pso,
                                lhsT=xt[:, ic, ti * P : (ti + 1) * P],
                                rhs=w_sb[ic][:, on * 512 : (on + 1) * 512],
                                start=(ic == 0),
                                stop=(ic == N_IC - 1),
                            )
                        nc.vector.tensor_add(
                            out=o_sb[:, on * 512 : (on + 1) * 512],
                            in0=pso,
                            in1=bias_sb[:, on * 512 : (on + 1) * 512],
                        )
                        if is_last:
                            # split the final store: each half leaves as soon
                            # as its bias add lands, on separate engines
                            eng = nc.scalar if on == 0 else nc.sync
                            eng.dma_start(
                                out[tok : tok + P, on * 512 : (on + 1) * 512],
                                o_sb[:, on * 512 : (on + 1) * 512],
                            )
                    if not is_last:
                        nc.scalar.dma_start(out[tok : tok + P, :], o_sb)

    nc.finalize()
    return nc


_NC = None


def _get_nc():
    global _NC
    if _NC is None:
        _NC = build_nc()
    return _NC


def kernel(**inputs):
    x = np.asarray(inputs["x"], dtype=np.float32)
    shared = {
        "WT": np.ascontiguousarray(np.asarray(inputs["W"], np.float32).T),
        "b": np.ascontiguousarray(np.asarray(inputs["b"], np.float32)),
        "A": np.ascontiguousarray(
            np.concatenate(
                [np.asarray(inputs["A1"], np.float32), np.asarray(inputs["A2"], np.float32)],
                axis=0,
            )
        ),
        "SC": np.asarray([SCALE1] * RANK + [SCALE2] * RANK, np.float32).reshape(
            RANK2, 1
        ),
        "BT": np.ascontiguousarray(
            np.concatenate(
                [np.asarray(inputs["B1"], np.float32).T, np.asarray(inputs["B2"], np.float32).T],
                axis=0,
            )
        ),
    }
    in_maps = []
    for c in range(N_CORES):
        m = dict(shared)
        m["xT"] = np.ascontiguousarray(x[c * T_SHARD : (c + 1) * T_SHARD].T)
        in_maps.append(m)
    res = run_bass_kernel_spmd(_get_nc(), in_maps, core_ids=list(range(N_CORES)))
    return np.concatenate(
        [np.asarray(r["out"]).astype(np.float32) for r in res.results], axis=0
    )


# revision 9
# speedup vs baseline: 1.1523x; 1.0017x over previous
"""LoRALinear kernel for Trainium2 (8 NeuronCores, data-parallel over tokens).

Math: out = x @ W.T + b + s1*(x@A1.T)@B1.T + s2*(x@A2.T)@B2.T
    = x @ (W + s1*B1@A1 + s2*B2@A2).T + b

The two rank-16 adapters are stacked into one rank-32 pair (host-side
concatenation, layout only) and folded into the base weight on-device,
turning the problem into one dense [T,1024]@[1024,1024] matmul plus a
broadcast bias add. x is sharded 4096 tokens per core; weights replicated.

Design notes (vs the v1 baseline, 148.3us -> 128.9us modeled):
  - No fp32->fp32r conversion copies: x/W/A live as fp32r end-to-end (the
    DMA moves the same fp32 bits; the PE consumes them in its 1-pass
    fp32r mode). Compute-produced matmul operands (scaled B^T, folded W)
    are written as fp32r by their producing op, which the BIR verifier
    requires ("rounded to FP32r").
  - W is DMA'd in 8 contraction chunks; each chunk's LoRA fold (rank-32
    matmul + in-place DVE add) runs as the chunk arrives, interleaved
    with the first x chunks, so the PE has work ~5us in. All transfers
    share one serial DMA pool, so the dma_start order below IS the
    schedule; it was tuned against the TimelineSim cost model.
  - PE warmup matmuls on a zeroed scratch tile absorb the slow-clock ramp
    while the first DMAs are in flight.
  - Per-partition-sliced engine ops must start at partition 0/32/64/96,
    so the two adapter scales (s1 rows 0..15, s2 rows 16..31) are one
    full-tile ACT op with a per-partition scale vector passed as a tiny
    constant input.
  - Output is stored as bf16 (adds ~1e-3 relative error against the 2e-2
    budget; halves store traffic) and exactly upcast on the host. Input
    DMAs issue on SP, stores on ACT, so a store waiting for its tile
    can't head-of-line block the input prefetch stream; the final store
    is split across both engines to shorten the tail.
"""

import sys

import numpy as np

try:
    import concourse.bass as bass
except ImportError:
    sys.path.insert(0, "/opt/trn_rl_repo")
    import concourse.bass as bass

from concourse import bacc

import concourse.mybir as mybir
import concourse.tile as tile
from concourse.bass_utils import run_bass_kernel_spmd

TOKENS, D, RANK = 32768, 1024, 16
RANK2 = 2 * RANK
N_CORES = 8
T_SHARD = TOKENS // N_CORES  # 4096
SCALE1 = 8.0 / RANK
SCALE2 = 16.0 / RANK
F32 = mybir.dt.float32
F32R = mybir.dt.float32r
BF16 = mybir.dt.bfloat16
P = 128
N_IC = D // P  # 8 contraction chunks
N_OC = D // 512  # 2 psum-wide output chunks
N_TT = T_SHARD // P  # 32 token tiles per core
COPY = mybir.ActivationFunctionType.Copy

# x prefetch chunk sizes (tokens); first ones small so the PE can start early
X_CHUNKS = [128, 128, 256, 256, 512, 512, 512, 512, 512, 512, 256]
assert sum(X_CHUNKS) == T_SHARD

N_WARM = 7  # PE warmup matmuls; end right when fold0's inputs land


def build_nc():
    nc = bacc.Bacc("TRN2")
    xT = nc.dram_tensor("xT", [D, T_SHARD], F32R, kind="ExternalInput")
    WT = nc.dram_tensor("WT", [D, D], F32R, kind="ExternalInput")
    b = nc.dram_tensor("b", [D], F32, kind="ExternalInput")
    A = nc.dram_tensor("A", [RANK2, D], F32R, kind="ExternalInput")
    BT = nc.dram_tensor("BT", [RANK2, D], F32, kind="ExternalInput")
    SC = nc.dram_tensor("SC", [RANK2, 1], F32, kind="ExternalInput")
    out = nc.dram_tensor("out", [T_SHARD, D], BF16, kind="ExternalOutput")

    with tile.TileContext(nc) as tc:
        with (
            tc.tile_pool(name="const", bufs=1) as const,
            tc.tile_pool(name="xp2", bufs=2) as xp2,
            tc.tile_pool(name="xp5", bufs=3) as xp5,
            tc.tile_pool(name="op", bufs=8) as opool,
            tc.tile_pool(name="psm", bufs=6, space="PSUM") as psum_m,
            tc.tile_pool(name="psf", bufs=2, space="PSUM") as psum_f,
        ):
            # --- PE warmup: matmuls on a zeroed scratch tile, results unused
            wz = const.tile([P, 512], F32)
            nc.vector.memset(wz, 0.0)
            warm = const.tile([P, 512], F32R)
            nc.vector.tensor_copy(out=warm, in_=wz)
            for i in range(N_WARM):
                pw = psum_f.tile([P, 512], F32, tag="fold")
                nc.tensor.matmul(
                    pw,
                    lhsT=warm[:, 0:P],
                    rhs=warm[:],
                    start=True,
                    stop=True,
                )

            # --- small constants: bias (partition-broadcast), adapters
            bias_sb = const.tile([P, D], F32)

            BT_ld = const.tile([RANK2, D], F32)
            nc.sync.dma_start(BT_ld, BT[:])
            A_sb = const.tile([RANK2, D], F32R)
            nc.sync.dma_start(A_sb, A[:])
            sc_sb = const.tile([RANK2, 1], F32)
            nc.sync.dma_start(sc_sb, SC[:])
            # one full-tile scale with a per-partition scalar vector
            # (partition-sliced engine ops must start at partition 0/32/64/96)
            BTs_sb = const.tile([RANK2, D], F32R)
            nc.vector.tensor_scalar_mul(BTs_sb, BT_ld, sc_sb[:])

            # --- W chunks + fold, interleaved with x prefetch.
            # One tile per contraction chunk; fold adds run on DVE in place.
            w_sb = [
                const.tile([P, D], F32R, name=f"w_sb{ic}") for ic in range(N_IC)
            ]

            x_tiles = []  # (tile, chunk_token_offset, n_tiles_in_chunk)

            def dma_w(ic):
                nc.sync.dma_start(w_sb[ic], WT[ic * P : (ic + 1) * P, :])
                for on in range(N_OC):
                    psd = psum_f.tile([P, 512], F32, tag="fold")
                    nc.tensor.matmul(
                        psd,
                        lhsT=A_sb[:, ic * P : (ic + 1) * P],
                        rhs=BTs_sb[:, on * 512 : (on + 1) * 512],
                        start=True,
                        stop=True,
                    )
                    nc.vector.tensor_add(
                        out=w_sb[ic][:, on * 512 : (on + 1) * 512],
                        in0=w_sb[ic][:, on * 512 : (on + 1) * 512].bitcast(F32),
                        in1=psd,
                    )

            tile_map = {}

            def dma_x(ci, t0, tn):
                pool = xp2 if tn < 512 else xp5
                xt = pool.tile([P, N_IC, tn], F32R, tag=f"x{tn}")
                nc.sync.dma_start(
                    xt,
                    xT[:, t0 : t0 + tn].rearrange("(io ii) t -> ii io t", ii=P),
                )
                x_tiles.append((xt, t0, tn // P))
                for k in range(tn // P):
                    tile_map[t0 // P + k] = (xt, k)

            # interleave: early x tiles feed partial psum groups while W streams
            t0 = 0
            ci = 0

            def next_x():
                nonlocal t0, ci
                dma_x(ci, t0, X_CHUNKS[ci])
                t0 += X_CHUNKS[ci]
                ci += 1

            # early psum groups, advanced one ic per W-chunk arrival so the
            # in-order PE queue always has a runnable matmul; a group opening
            # at slice k first takes all already-available ics <= k
            early = []

            def open_early(tt, on):
                pso = psum_m.tile([P, 512], F32, tag="m", name=f"eg{tt}_{on}")
                early.append(
                    {"tt": tt, "on": on, "pso": pso, "ics": [], "started": False}
                )

            def emit_early(g, ic, stop=False):
                xt, ti = tile_map[g["tt"]]
                nc.tensor.matmul(
                    g["pso"],
                    lhsT=xt[:, ic, ti * P : (ti + 1) * P],
                    rhs=w_sb[ic][:, g["on"] * 512 : (g["on"] + 1) * 512],
                    start=not g["started"],
                    stop=stop,
                )
                g["started"] = True
                g["ics"].append(ic)

            def do_slice(k):
                for g in early:
                    for ic in range(k + 1):
                        if ic not in g["ics"]:
                            emit_early(
                                g,
                                ic,
                                stop=(ic == N_IC - 1 and len(g["ics"]) == N_IC - 1),
                            )

            dma_w(0)
            next_x()  # 128 tokens -> tile 0
            open_early(0, 0)
            open_early(0, 1)
            do_slice(0)
            dma_w(1)
            next_x()  # 128 tokens -> tile 1
            open_early(1, 0)
            open_early(1, 1)
            do_slice(1)
            dma_w(2)
            next_x()  # 256 tokens -> tiles 2,3
            do_slice(2)
            dma_w(3)
            do_slice(3)
            dma_w(4)
            open_early(2, 0)
            open_early(2, 1)
            do_slice(4)
            dma_w(5)
            do_slice(5)
            dma_w(6)
            do_slice(6)
            dma_w(7)
            do_slice(7)
            b_ap = b[:]
            bias_bcast = bass.AP(
                tensor=b_ap.tensor, offset=b_ap.offset, ap=[[0, P], [1, D]]
            )
            nc.sync.dma_start(out=bias_sb, in_=bias_bcast)
            while ci < len(X_CHUNKS):
                next_x()

            # drain the early groups; stores deferred past the input stream
            deferred = []
            for g in early:
                o_e = opool.tile([P, 512], BF16, tag="oe", bufs=6, name="o_e")
                on = g["on"]
                nc.vector.tensor_add(
                    out=o_e, in0=g["pso"], in1=bias_sb[:, on * 512 : (on + 1) * 512]
                )
                deferred.append(
                    (
                        out[g["tt"] * P : (g["tt"] + 1) * P, on * 512 : (on + 1) * 512],
                        o_e,
                    )
                )

            # --- main loop: 128-token psum groups, 8 accumulating matmuls each
            last_tok = T_SHARD - P
            done_tiles = {g["tt"] for g in early}
            for xt, t0, ntile in x_tiles:
                for ti in range(ntile):
                    tok = t0 + ti * P
                    if tok // P in done_tiles:
                        continue
                    o_sb = opool.tile([P, D], BF16, tag="o")
                    is_last = tok == last_tok
                    for on in range(N_OC):
                        pso = psum_m.tile([P, 512], F32, tag="m")
                        for ic in range(N_IC):
                            nc.tensor.matmul(
                                pso,
                                lhsT=xt[:, ic, ti * P : (ti + 1) * P],
                                rhs=w_sb[ic][:, on * 512 : (on + 1) * 512],
                                start=(ic == 0),
                                stop=(ic == N_IC - 1),
                            )
                        nc.vector.tensor_add(
                            out=o_sb[:, on * 512 : (on + 1) * 512],
                            in0=pso,
                            in1=bias_sb[:, on * 512 : (on + 1) * 512],
                        )
                        if is_last:
                            # split the final store: each half leaves as soon
                            # as its bias add lands, on separate engines
                            eng = nc.scalar if on == 0 else nc.sync
                            eng.dma_start(
                                out[tok : tok + P, on * 512 : (on + 1) * 512],
                                o_sb[:, on * 512 : (on + 1) * 512],
                            )
                    if not is_last:
                        nc.scalar.dma_start(out[tok : tok + P, :], o_sb)
                    if deferred:
                        nc.scalar.dma_start(*deferred.pop(0))

    nc.finalize()
    return nc


_NC = None


def _get_nc():
    global _NC
    if _NC is None:
        _NC = build_nc()
    return _NC


def kernel(**inputs):
    x = np.asarray(inputs["x"], dtype=np.float32)
    shared = {
        "WT": np.ascontiguousarray(np.asarray(inputs["W"], np.float32).T),
        "b": np.ascontiguousarray(np.asarray(inputs["b"], np.float32)),
        "A": np.ascontiguousarray(
            np.concatenate(
                [np.asarray(inputs["A1"], np.float32), np.asarray(inputs["A2"], np.float32)],
                axis=0,
            )
        ),
        "SC": np.asarray([SCALE1] * RANK + [SCALE2] * RANK, np.float32).reshape(
            RANK2, 1
        ),
        "BT": np.ascontiguousarray(
            np.concatenate(
                [np.asarray(inputs["B1"], np.float32).T, np.asarray(inputs["B2"], np.float32).T],
                axis=0,
            )
        ),
    }
    in_maps = []
    for c in range(N_CORES):
        m = dict(shared)
        m["xT"] = np.ascontiguousarray(x[c * T_SHARD : (c + 1) * T_SHARD].T)
        in_maps.append(m)
    res = run_bass_kernel_spmd(_get_nc(), in_maps, core_ids=list(range(N_CORES)))
    return np.concatenate(
        [np.asarray(r["out"]).astype(np.float32) for r in res.results], axis=0
    )


# revision 11
# speedup vs baseline: 1.1643x; 1.0104x over previous
"""LoRALinear kernel for Trainium2 (8 NeuronCores, data-parallel over tokens).

Math: out = x @ W.T + b + s1*(x@A1.T)@B1.T + s2*(x@A2.T)@B2.T
    = x @ (W + s1*B1@A1 + s2*B2@A2).T + b

The two rank-16 adapters are stacked into one rank-32 pair (host-side
concatenation, layout only) and folded into the base weight on-device,
turning the problem into one dense [T,1024]@[1024,1024] matmul plus a
broadcast bias add. x is sharded 4096 tokens per core; weights replicated.

Design notes (vs the v1 baseline, 148.3us -> 128.7us modeled):
  - No fp32->fp32r conversion copies: x/W/A live as fp32r end-to-end (the
    DMA moves the same fp32 bits; the PE consumes them in its 1-pass
    fp32r mode). Compute-produced matmul operands (scaled B^T, folded W)
    are written as fp32r by their producing op, which the BIR verifier
    requires ("rounded to FP32r").
  - W is DMA'd in 8 contraction chunks; each chunk's LoRA fold (rank-32
    matmul + in-place DVE add) runs as the chunk arrives, interleaved
    with the first x chunks, so the PE has work ~5us in. All transfers
    share one serial DMA pool, so the dma_start order below IS the
    schedule; it was tuned against the TimelineSim cost model.
  - PE warmup matmuls on a zeroed scratch tile absorb the slow-clock ramp
    while the first DMAs are in flight.
  - Per-partition-sliced engine ops must start at partition 0/32/64/96,
    so the two adapter scales (s1 rows 0..15, s2 rows 16..31) are one
    full-tile DVE op with a per-partition scale vector passed as a tiny
    constant input. DVE, not ACT: the first ACT compute op triggers a
    1.3us activation-table load that would gate the first fold.
  - Early psum groups are advanced one ic per W-chunk arrival ("ic-sliced"
    emission; accumulation order within a psum group is free) so the
    in-order PE queue always has a runnable matmul while W streams; their
    stores are deferred past the input-heavy window and drip out later.
  - Output is stored as bf16 (adds ~1e-3 relative error against the 2e-2
    budget; halves store traffic) and exactly upcast on the host. Input
    DMAs issue on SP, stores on ACT, so a store waiting for its tile
    can't head-of-line block the input prefetch stream; the final store
    is split across both engines to shorten the tail.
"""

import sys

import numpy as np

try:
    import concourse.bass as bass
except ImportError:
    sys.path.insert(0, "/opt/trn_rl_repo")
    import concourse.bass as bass

from concourse import bacc

import concourse.mybir as mybir
import concourse.tile as tile
from concourse.bass_utils import run_bass_kernel_spmd

TOKENS, D, RANK = 32768, 1024, 16
RANK2 = 2 * RANK
N_CORES = 8
T_SHARD = TOKENS // N_CORES  # 4096
SCALE1 = 8.0 / RANK
SCALE2 = 16.0 / RANK
F32 = mybir.dt.float32
F32R = mybir.dt.float32r
BF16 = mybir.dt.bfloat16
P = 128
N_IC = D // P  # 8 contraction chunks
N_OC = D // 512  # 2 psum-wide output chunks
N_TT = T_SHARD // P  # 32 token tiles per core
COPY = mybir.ActivationFunctionType.Copy

# x prefetch chunk sizes (tokens); first ones small so the PE can start early
X_CHUNKS = [128, 128, 256, 256, 512, 512, 512, 512, 512, 512, 256]
assert sum(X_CHUNKS) == T_SHARD

N_WARM = 7  # PE warmup matmuls; end right when fold0's inputs land


def build_nc():
    nc = bacc.Bacc("TRN2")
    xT = nc.dram_tensor("xT", [D, T_SHARD], F32R, kind="ExternalInput")
    WT = nc.dram_tensor("WT", [D, D], F32R, kind="ExternalInput")
    b = nc.dram_tensor("b", [D], F32, kind="ExternalInput")
    CST = nc.dram_tensor("CST", [RANK2, 2 * D + 1], F32R, kind="ExternalInput")
    out = nc.dram_tensor("out", [T_SHARD, D], BF16, kind="ExternalOutput")

    with tile.TileContext(nc) as tc:
        with (
            tc.tile_pool(name="const", bufs=1) as const,
            tc.tile_pool(name="xp2", bufs=2) as xp2,
            tc.tile_pool(name="xp5", bufs=3) as xp5,
            tc.tile_pool(name="op", bufs=8) as opool,
            tc.tile_pool(name="psm", bufs=6, space="PSUM") as psum_m,
            tc.tile_pool(name="psf", bufs=2, space="PSUM") as psum_f,
        ):
            # --- PE warmup: matmuls on a zeroed scratch tile, results unused
            wz = const.tile([P, 512], F32)
            nc.vector.memset(wz, 0.0)
            warm = const.tile([P, 512], F32R)
            nc.vector.tensor_copy(out=warm, in_=wz)
            for i in range(N_WARM):
                pw = psum_f.tile([P, 512], F32, tag="fold")
                nc.tensor.matmul(
                    pw,
                    lhsT=warm[:, 0:P],
                    rhs=warm[:],
                    start=True,
                    stop=True,
                )

            # --- small constants: bias (partition-broadcast), adapters
            bias_sb = const.tile([P, D], F32)

            # B^T, A and the scale vector arrive as ONE DMA: back-to-back
            # small transfers would each pace at the 0.625us HWDGE stage,
            # not their ~0.4us transfer, delaying the whole W stream
            cst = const.tile([RANK2, 2 * D + 1], F32R)
            nc.sync.dma_start(cst, CST[:])
            A_sb = cst[:, D : 2 * D]
            # one full-tile scale with a per-partition scalar vector
            # (partition-sliced engine ops must start at partition 0/32/64/96)
            BTs_sb = const.tile([RANK2, D], F32R)
            nc.vector.tensor_scalar_mul(
                BTs_sb, cst[:, 0:D].bitcast(F32), cst[:, 2 * D : 2 * D + 1].bitcast(F32)
            )

            # --- W chunks + fold, interleaved with x prefetch.
            # One tile per contraction chunk; fold adds run on DVE in place.
            w_sb = [
                const.tile([P, D], F32R, name=f"w_sb{ic}") for ic in range(N_IC)
            ]

            x_tiles = []  # (tile, chunk_token_offset, n_tiles_in_chunk)

            def dma_w(ic):
                nc.sync.dma_start(w_sb[ic], WT[ic * P : (ic + 1) * P, :])
                for on in range(N_OC):
                    psd = psum_f.tile([P, 512], F32, tag="fold")
                    nc.tensor.matmul(
                        psd,
                        lhsT=A_sb[:, ic * P : (ic + 1) * P],
                        rhs=BTs_sb[:, on * 512 : (on + 1) * 512],
                        start=True,
                        stop=True,
                    )
                    nc.vector.tensor_add(
                        out=w_sb[ic][:, on * 512 : (on + 1) * 512],
                        in0=w_sb[ic][:, on * 512 : (on + 1) * 512].bitcast(F32),
                        in1=psd,
                    )

            tile_map = {}

            def dma_x(ci, t0, tn):
                pool = xp2 if tn < 512 else xp5
                xt = pool.tile([P, N_IC, tn], F32R, tag=f"x{tn}")
                nc.sync.dma_start(
                    xt,
                    xT[:, t0 : t0 + tn].rearrange("(io ii) t -> ii io t", ii=P),
                )
                x_tiles.append((xt, t0, tn // P))
                for k in range(tn // P):
                    tile_map[t0 // P + k] = (xt, k)

            # interleave: early x tiles feed partial psum groups while W streams
            t0 = 0
            ci = 0

            def next_x():
                nonlocal t0, ci
                dma_x(ci, t0, X_CHUNKS[ci])
                t0 += X_CHUNKS[ci]
                ci += 1

            # early psum groups, advanced one ic per W-chunk arrival so the
            # in-order PE queue always has a runnable matmul; a group opening
            # at slice k first takes all already-available ics <= k
            early = []

            def open_early(tt, on):
                pso = psum_m.tile([P, 512], F32, tag="m", name=f"eg{tt}_{on}")
                early.append(
                    {"tt": tt, "on": on, "pso": pso, "ics": [], "started": False}
                )

            def emit_early(g, ic, stop=False):
                xt, ti = tile_map[g["tt"]]
                nc.tensor.matmul(
                    g["pso"],
                    lhsT=xt[:, ic, ti * P : (ti + 1) * P],
                    rhs=w_sb[ic][:, g["on"] * 512 : (g["on"] + 1) * 512],
                    start=not g["started"],
                    stop=stop,
                )
                g["started"] = True
                g["ics"].append(ic)

            def do_slice(k):
                for g in early:
                    for ic in range(k + 1):
                        if ic not in g["ics"]:
                            emit_early(
                                g,
                                ic,
                                stop=(ic == N_IC - 1 and len(g["ics"]) == N_IC - 1),
                            )

            dma_w(0)
            next_x()  # 128 tokens -> tile 0
            open_early(0, 0)
            open_early(0, 1)
            do_slice(0)
            dma_w(1)
            next_x()  # 128 tokens -> tile 1
            open_early(1, 0)
            open_early(1, 1)
            do_slice(1)
            dma_w(2)
            next_x()  # 256 tokens -> tiles 2,3
            do_slice(2)
            dma_w(3)
            do_slice(3)
            dma_w(4)
            open_early(2, 0)
            open_early(2, 1)
            do_slice(4)
            dma_w(5)
            do_slice(5)
            dma_w(6)
            do_slice(6)
            dma_w(7)
            do_slice(7)
            b_ap = b[:]
            bias_bcast = bass.AP(
                tensor=b_ap.tensor, offset=b_ap.offset, ap=[[0, P], [1, D]]
            )
            nc.sync.dma_start(out=bias_sb, in_=bias_bcast)
            while ci < len(X_CHUNKS):
                next_x()

            # drain the early groups; stores deferred past the input stream
            deferred = []
            for g in early:
                o_e = opool.tile([P, 512], BF16, tag="oe", bufs=6, name="o_e")
                on = g["on"]
                nc.vector.tensor_add(
                    out=o_e, in0=g["pso"], in1=bias_sb[:, on * 512 : (on + 1) * 512]
                )
                deferred.append(
                    (
                        out[g["tt"] * P : (g["tt"] + 1) * P, on * 512 : (on + 1) * 512],
                        o_e,
                    )
                )

            # --- main loop: 128-token psum groups, 8 accumulating matmuls each
            last_tok = T_SHARD - P
            done_tiles = {g["tt"] for g in early}
            for xt, t0, ntile in x_tiles:
                for ti in range(ntile):
                    tok = t0 + ti * P
                    if tok // P in done_tiles:
                        continue
                    o_sb = opool.tile([P, D], BF16, tag="o")
                    is_last = tok == last_tok
                    for on in range(N_OC):
                        pso = psum_m.tile([P, 512], F32, tag="m")
                        for ic in range(N_IC):
                            nc.tensor.matmul(
                                pso,
                                lhsT=xt[:, ic, ti * P : (ti + 1) * P],
                                rhs=w_sb[ic][:, on * 512 : (on + 1) * 512],
                                start=(ic == 0),
                                stop=(ic == N_IC - 1),
                            )
                        nc.vector.tensor_add(
                            out=o_sb[:, on * 512 : (on + 1) * 512],
                            in0=pso,
                            in1=bias_sb[:, on * 512 : (on + 1) * 512],
                        )
                        if is_last:
                            # split the final store: each half leaves as soon
                            # as its bias add lands, on separate engines
                            eng = nc.scalar if on == 0 else nc.sync
                            eng.dma_start(
                                out[tok : tok + P, on * 512 : (on + 1) * 512],
                                o_sb[:, on * 512 : (on + 1) * 512],
                            )
                    if not is_last:
                        nc.scalar.dma_start(out[tok : tok + P, :], o_sb)
                    if deferred:
                        nc.scalar.dma_start(*deferred.pop(0))

    nc.finalize()
    return nc


_NC = None


def _get_nc():
    global _NC
    if _NC is None:
        _NC = build_nc()
    return _NC


def kernel(**inputs):
    x = np.asarray(inputs["x"], dtype=np.float32)
    shared = {
        "WT": np.ascontiguousarray(np.asarray(inputs["W"], np.float32).T),
        "b": np.ascontiguousarray(np.asarray(inputs["b"], np.float32)),
        "CST": np.ascontiguousarray(
            np.concatenate(
                [
                    np.concatenate(
                        [
                            np.asarray(inputs["B1"], np.float32).T,
                            np.asarray(inputs["B2"], np.float32).T,
                        ],
                        axis=0,
                    ),
                    np.concatenate(
                        [
                            np.asarray(inputs["A1"], np.float32),
                            np.asarray(inputs["A2"], np.float32),
                        ],
                        axis=0,
                    ),
                    np.asarray([SCALE1] * RANK + [SCALE2] * RANK, np.float32).reshape(
                        RANK2, 1
                    ),
                ],
                axis=1,
            )
        ),
    }
    in_maps = []
    for c in range(N_CORES):
        m = dict(shared)
        m["xT"] = np.ascontiguousarray(x[c * T_SHARD : (c + 1) * T_SHARD].T)
        in_maps.append(m)
    res = run_bass_kernel_spmd(_get_nc(), in_maps, core_ids=list(range(N_CORES)))
    return np.concatenate(
        [np.asarray(r["out"]).astype(np.float32) for r in res.results], axis=0
    )
